# revision 8
# baseline (speedup 1.0000x reference)
"""GCNN message-passing layer on 8 Trainium2 NeuronCores (Bass/Tile).

Math (per token m, all within one sentence of L=64 tokens):
    in_pot[m]  = (rep @ W_in)[head(m)] + b_in[lab(m)]
    in_gate[m] = (rep @ W_gate_in)[head(m)] + b_gate_in[lab(m)]
    self_pot   = rep @ W_self ; self_gate = rep @ W_gate_self
    w_d = sigmoid(gate_d) * msoft_d^2
    out = relu(in_pot*w_in + self_pot*w_self) * mask

Sharding: data-parallel over BNK (160 sentences / core). All gathers stay
within a sentence, so shards are independent; weights are replicated.

Device strategy per 128-token tile (2 sentences):
  - rep arrives host-pretransposed as repT chunks so DIN sits on partitions.
  - One fused matmul produces [proj_in | gate_in | gate_self]; another W_self.
  - The within-tile head gather is a matmul with a host-built one-hot scatter
    matrix; the relation-bias lookup is a matmul with a one-hot label matrix,
    accumulated into the same PSUM tile. float32r streams fp32 data at full
    PE rate for free dims >= 256.
  - Gate weighting/masking runs on ACT/DVE straight out of PSUM.
"""

import numpy as np

import concourse.bass as bass
import concourse.mybir as mybir
import concourse.tile as tile
from concourse import bacc, bass_utils

BNK, L, DIN, DOUT, NREL = 1280, 64, 512, 256, 40
NCORES = 8
SPC = BNK // NCORES          # sentences per core
TOK = SPC * L                # tokens per core (10240)
TILE_T = 128                 # tokens per device tile
KC = DIN // 128              # K chunks (4)
NTILES = TOK // TILE_T       # 80

F32 = mybir.dt.float32
F32R = mybir.dt.float32r
AF = mybir.ActivationFunctionType
ALU = mybir.AluOpType


def build_nc(ntiles: int = NTILES):
    """Build the per-core Bass program (same program on all cores)."""
    tok = ntiles * TILE_T
    nc = bacc.Bacc("TRN2", target_bir_lowering=False, debug=False)

    # --- DRAM I/O -------------------------------------------------------
    repT_d = nc.dram_tensor("repT", [ntiles, 128, KC, TILE_T], F32R, kind="ExternalInput")
    scatH_d = nc.dram_tensor("scatH", [ntiles, TILE_T, TILE_T], F32R, kind="ExternalInput")
    scatL_d = nc.dram_tensor("scatL", [ntiles, NREL, TILE_T], F32R, kind="ExternalInput")
    wa_d = nc.dram_tensor("wa", [128, KC, DOUT + 2], F32R, kind="ExternalInput")
    ws_d = nc.dram_tensor("ws", [128, KC, DOUT], F32R, kind="ExternalInput")
    ball_d = nc.dram_tensor("ball", [NREL, DOUT + 2], F32R, kind="ExternalInput")
    aux_d = nc.dram_tensor("aux", [128, ntiles, 2], F32, kind="ExternalInput")
    out_d = nc.dram_tensor("out", [tok, DOUT], F32, kind="ExternalOutput")

    with tile.TileContext(nc) as tc:
        with (
            tc.tile_pool(name="const", bufs=1) as const_pool,
            tc.tile_pool(name="rep", bufs=4) as rep_pool,
            tc.tile_pool(name="scat", bufs=4) as scat_pool,
            tc.tile_pool(name="src", bufs=3) as src_pool,
            tc.tile_pool(name="small", bufs=4) as small_pool,
            tc.tile_pool(name="big", bufs=3) as big_pool,
            tc.tile_pool(name="psum", bufs=2, space="PSUM") as psum_pool,
        ):
            # Resident constants
            wa_sb = const_pool.tile([128, KC, DOUT + 2], F32R)
            nc.sync.dma_start(wa_sb[:], wa_d[:])
            ws_sb = const_pool.tile([128, KC, DOUT], F32R)
            nc.sync.dma_start(ws_sb[:], ws_d[:])
            ball_sb = const_pool.tile([NREL, DOUT + 2], F32R)
            nc.sync.dma_start(ball_sb[:], ball_d[:])
            aux_sb = const_pool.tile([128, ntiles, 2], F32)
            nc.sync.dma_start(aux_sb[:], aux_d[:])

            for i in range(ntiles):
                rep_sb = rep_pool.tile([128, KC, TILE_T], F32R)
                nc.sync.dma_start(rep_sb[:], repT_d[i])
                scath_sb = scat_pool.tile([TILE_T, TILE_T], F32R, tag="scath")
                nc.sync.dma_start(scath_sb[:], scatH_d[i])
                scatl_sb = scat_pool.tile([NREL, TILE_T], F32R, tag="scatl")
                nc.sync.dma_start(scatl_sb[:], scatL_d[i])

                # [proj_in | gate_in | gate_self] and self potential
                psum_a = psum_pool.tile([128, DOUT + 2], F32, tag="pa")
                psum_b = psum_pool.tile([128, DOUT], F32, tag="pb")
                for kc in range(KC):
                    first, last = kc == 0, kc == KC - 1
                    nc.tensor.matmul(psum_a[:], rep_sb[:, kc, :], wa_sb[:, kc, :],
                                     start=first, stop=last)
                    nc.tensor.matmul(psum_b[:], rep_sb[:, kc, :], ws_sb[:, kc, :],
                                     start=first, stop=last)

                # head-gather + relation bias via scatter matmuls
                # (258 wide: fp32r matmul dst free size must be even; the
                # last column gathers gate_self and is unused)
                src_sb = src_pool.tile([128, DOUT + 2], F32R)
                nc.scalar.activation(src_sb[:], psum_a[:, 0:DOUT + 2], AF.Copy)
                psum_g = psum_pool.tile([128, DOUT + 2], F32, tag="pg")
                nc.tensor.matmul(psum_g[:], scath_sb[:], src_sb[:], start=True, stop=False)
                nc.tensor.matmul(psum_g[:], scatl_sb[:], ball_sb[:], start=False, stop=True)

                # gate weights: sigmoid(gate) * msoft^2 * mask
                w_in = small_pool.tile([128, 1], F32, tag="w_in")
                nc.scalar.activation(w_in[:], psum_g[:, DOUT:DOUT + 1], AF.Sigmoid)
                w_self = small_pool.tile([128, 1], F32, tag="w_self")
                nc.scalar.activation(w_self[:], psum_a[:, DOUT + 1:DOUT + 2], AF.Sigmoid)
                w_in_f = small_pool.tile([128, 1], F32, tag="w_in_f")
                nc.vector.tensor_mul(w_in_f[:], w_in[:], aux_sb[:, i, 0:1])
                w_self_f = small_pool.tile([128, 1], F32, tag="w_self_f")
                nc.vector.tensor_mul(w_self_f[:], w_self[:], aux_sb[:, i, 1:2])

                # res = relu(in_pot*w_in + self_pot*w_self)
                t_sb = big_pool.tile([128, DOUT], F32, tag="t")
                nc.vector.tensor_scalar_mul(t_sb[:], psum_g[:, 0:DOUT], w_in_f[:])
                u_sb = big_pool.tile([128, DOUT], F32, tag="u")
                nc.vector.scalar_tensor_tensor(u_sb[:], psum_b[:], w_self_f[:], t_sb[:],
                                               op0=ALU.mult, op1=ALU.add)
                o_sb = big_pool.tile([128, DOUT], F32, tag="o")
                nc.scalar.activation(o_sb[:], u_sb[:], AF.Relu)
                nc.sync.dma_start(out_d[bass.ts(i, TILE_T), :], o_sb[:])

    nc.compile()
    return nc


def prep_core_inputs(c, rep, adj_arc, adj_lab, adj_mask_in, adj_mask_loop, mask,
                     Wa, Ws, ball, ntiles: int = NTILES):
    """Build the per-core in_map (host-side shard + layout prep)."""
    tok = ntiles * TILE_T
    sh = slice(c * SPC, (c + 1) * SPC)
    rep_s = np.ascontiguousarray(rep[sh]).reshape(SPC * L, DIN)[:tok]
    x = rep_s.reshape(ntiles, TILE_T, KC, 128)
    repT = np.ascontiguousarray(x.transpose(0, 3, 2, 1))  # [tile, k, kc, t]

    sent = adj_arc[sh, :, 0].reshape(-1)[:tok].astype(np.int64)
    head = adj_arc[sh, :, 1].reshape(-1)[:tok].astype(np.int64)
    idx_local = sent * L + head - c * SPC * L
    t_all = np.arange(tok)
    if idx_local.min() < 0 or idx_local.max() >= tok or np.any(idx_local // TILE_T != t_all // TILE_T):
        raise ValueError("head gather escapes its 128-token tile; unsupported input structure")
    lab = adj_lab[sh].reshape(-1)[:tok].astype(np.int64)

    scatH = np.zeros((ntiles, TILE_T, TILE_T), np.float32)
    scatH[t_all // TILE_T, idx_local % TILE_T, t_all % TILE_T] = 1.0
    scatL = np.zeros((ntiles, NREL, TILE_T), np.float32)
    scatL[t_all // TILE_T, lab, t_all % TILE_T] = 1.0

    msq_in = (adj_mask_in[sh] ** 2 * mask[sh]).reshape(-1)[:tok].astype(np.float32)
    msq_loop = (adj_mask_loop[sh] ** 2 * mask[sh]).reshape(-1)[:tok].astype(np.float32)
    aux = np.ascontiguousarray(
        np.stack([msq_in.reshape(ntiles, TILE_T).T, msq_loop.reshape(ntiles, TILE_T).T], axis=-1)
    )  # [128, ntiles, 2]

    return {"repT": repT, "scatH": scatH, "scatL": scatL,
            "wa": Wa, "ws": Ws, "ball": ball, "aux": aux}


def prep_shared(W_in, b_in, W_gate_in, b_gate_in, W_self, W_gate_self):
    Wa = np.concatenate([W_in, W_gate_in, W_gate_self], axis=1).astype(np.float32)
    Wa = np.ascontiguousarray(Wa.reshape(KC, 128, DOUT + 2).transpose(1, 0, 2))
    Ws = np.ascontiguousarray(W_self.astype(np.float32).reshape(KC, 128, DOUT).transpose(1, 0, 2))
    ball = np.ascontiguousarray(np.concatenate(
        [b_in, b_gate_in, np.zeros((NREL, 1), np.float32)], axis=1).astype(np.float32))
    return Wa, Ws, ball


_NC_CACHE = {}


def kernel(rep, adj_mask_in, adj_mask_loop, mask, W_in, b_in, W_gate_in,
           b_gate_in, W_self, W_gate_self, adj_arc_in, adj_lab_in):
    rep = np.asarray(rep, dtype=np.float32)
    Wa, Ws, ball = prep_shared(np.asarray(W_in), np.asarray(b_in), np.asarray(W_gate_in),
                               np.asarray(b_gate_in), np.asarray(W_self), np.asarray(W_gate_self))
    adj_arc = np.asarray(adj_arc_in)
    adj_lab = np.asarray(adj_lab_in)
    in_maps = [
        prep_core_inputs(c, rep, adj_arc, adj_lab, np.asarray(adj_mask_in),
                         np.asarray(adj_mask_loop), np.asarray(mask), Wa, Ws, ball)
        for c in range(NCORES)
    ]

    if "nc" not in _NC_CACHE:
        _NC_CACHE["nc"] = build_nc()
    nc = _NC_CACHE["nc"]

    res = bass_utils.run_bass_kernel_spmd(nc, in_maps, core_ids=list(range(NCORES)))
    out = np.concatenate([r["out"].reshape(SPC, L, DOUT) for r in res.results], axis=0)
    return out


# revision 20
# speedup vs baseline: 2.1863x; 2.1863x over previous
"""GCNN message-passing layer on 8 Trainium2 NeuronCores (Bass/Tile).

Math (per token m, all within one sentence of L=64 tokens):
    in_pot[m]  = (rep @ W_in)[head(m)] + b_in[lab(m)]
    in_gate[m] = (rep @ W_gate_in)[head(m)] + b_gate_in[lab(m)]
    self_pot   = rep @ W_self ; self_gate = rep @ W_gate_self
    w_d = sigmoid(gate_d) * msoft_d^2
    out = relu(in_pot*w_in + self_pot*w_self) * mask

Sharding: data-parallel over BNK (160 sentences / core). All gathers stay
within a sentence, so shards are independent; weights are replicated.

Device strategy per 128-token tile (2 sentences):
  - rep arrives host-pretransposed (fp16) so DIN sits on partitions.
  - One fused matmul produces [proj_in | gate_in | gate_self]; another W_self.
  - The within-tile head gather is a matmul with a host-built one-hot scatter
    matrix; the relation-bias lookup is a matmul with a one-hot label matrix
    accumulated into the same PSUM tile (skipped when b_in==0 and
    b_gate_in==1, which setup_inputs always produces - then the gate bias
    folds into the sigmoid's bias operand).
  - Gate weighting/masking runs on ACT/DVE straight out of PSUM; relu on
    GpSimd (otherwise idle). Output DMAs ride the second HWDGE ring (ACT's)
    so input and output streams don't serialize on one ring.
"""

import numpy as np

import concourse.bass as bass
import concourse.dve_ops as dve_ops
import concourse.mybir as mybir
import concourse.tile as tile
from concourse import bacc, bass_utils
from concourse.dve_spec import C0, C1, Spec, Src0, Src1, lower as dve_lower, relu as dve_relu
from concourse.dve_uop import DveOpSpec


def _register_gated_relu_op():
    """Register a fused custom-DVE op: out = relu(in0*s0 + in1*s1).

    Replaces the three stock DVE/ACT ops of the output tail (scale, fused
    multiply-add, relu) with a single Vector instruction. The microcode is
    lowered from the Spec at trace time like the stock custom ops; only the
    opcode row and sha pin need registering.
    """
    name = "GCNN_GATED_RELU_ANT"
    for op in dve_ops.OPS:
        if op.name == name:
            return op
    spec = Spec(
        body=dve_relu(Src0 * C0 + Src1 * C1),
        reference=lambda in0, in1, s0, s1, imm2: np.maximum(
            np.nan_to_num(in0.astype(np.float32) * s0 + in1 * s1,
                          nan=0.0, posinf=np.inf, neginf=-np.inf), 0.0),
    )
    row = dve_ops._CUSTOM_DVE_ROW_BASE + len(dve_ops.OPS)
    dve_ops._SUB_OPCODE_FOR_NAME[name] = row
    shas = {}
    for ver in ("v3", "v4"):
        uops = dve_lower(spec, ver=ver)
        shas[ver] = DveOpSpec(name=name, opcode=row, uops=uops, rd1_en=True).sha(ver)
    op = dve_ops.DveOp(name, spec, subdim=False, uops_sha=shas)
    dve_ops.OPS.append(op)
    dve_ops.CUSTOM_DVE_SPECS[name] = spec
    return op


GATED_RELU = _register_gated_relu_op()

BNK, L, DIN, DOUT, NREL = 1280, 64, 512, 256, 40
NCORES = 8
SPC = BNK // NCORES          # sentences per core
TOK = SPC * L                # tokens per core (10240)
TILE_T = 128                 # tokens per device tile
KC = DIN // 128              # K chunks (4)
NTILES = TOK // TILE_T       # 80
GROUP = 8                    # tiles per DMA batch

F32 = mybir.dt.float32
F16 = mybir.dt.float16
NP_MM = np.float16
AF = mybir.ActivationFunctionType
ALU = mybir.AluOpType


def build_nc(ntiles: int = NTILES, lab_bias: bool = True, gate_bias_one: bool = False):
    """Build the per-core Bass program (same program on all cores).

    lab_bias=False drops the relation-bias gather (valid when b_in is all
    zero); gate_bias_one then adds the constant 1.0 b_gate_in bias inside
    the sigmoid.
    """
    assert ntiles % GROUP == 0
    ngroups = ntiles // GROUP
    tok = ntiles * TILE_T
    nc = bacc.Bacc("TRN2", target_bir_lowering=False, debug=False)

    # --- DRAM I/O (DMA-batched by groups of GROUP tiles) ----------------
    repT_d = nc.dram_tensor("repT", [ngroups, 128, GROUP, KC, TILE_T], F16, kind="ExternalInput")
    scatH_d = nc.dram_tensor("scatH", [ngroups, TILE_T, GROUP, TILE_T], F16, kind="ExternalInput")
    if lab_bias:
        scatL_d = nc.dram_tensor("scatL", [ngroups, NREL, GROUP, TILE_T], F16, kind="ExternalInput")
        ball_d = nc.dram_tensor("ball", [NREL, DOUT + 2], F16, kind="ExternalInput")
    wa_d = nc.dram_tensor("wa", [128, KC, DOUT + 2], F16, kind="ExternalInput")
    ws_d = nc.dram_tensor("ws", [128, KC, DOUT], F16, kind="ExternalInput")
    aux_d = nc.dram_tensor("aux", [128, ntiles, 2], F32, kind="ExternalInput")
    out_d = nc.dram_tensor("out", [tok, DOUT], F32, kind="ExternalOutput")

    with tile.TileContext(nc) as tc:
        with (
            tc.tile_pool(name="const", bufs=1) as const_pool,
            tc.tile_pool(name="rep", bufs=3) as rep_pool,
            tc.tile_pool(name="scat", bufs=3) as scat_pool,
            tc.tile_pool(name="src", bufs=4) as src_pool,
            tc.tile_pool(name="small", bufs=8) as small_pool,
            tc.tile_pool(name="big", bufs=6) as big_pool,
            tc.tile_pool(name="out", bufs=3) as out_pool,
            tc.tile_pool(name="psum", bufs=3, space="PSUM") as psum_pool,
            tc.tile_pool(name="psum2", bufs=2, space="PSUM") as psum2_pool,
        ):
            # Resident constants
            wa_sb = const_pool.tile([128, KC, DOUT + 2], F16)
            nc.sync.dma_start(wa_sb[:], wa_d[:])
            ws_sb = const_pool.tile([128, KC, DOUT], F16)
            nc.sync.dma_start(ws_sb[:], ws_d[:])
            if lab_bias:
                ball_sb = const_pool.tile([NREL, DOUT + 2], F16)
                nc.sync.dma_start(ball_sb[:], ball_d[:])
            aux_sb = const_pool.tile([128, ntiles, 2], F32)
            nc.sync.dma_start(aux_sb[:], aux_d[:])

            for g in range(ngroups):
                rep_sb = rep_pool.tile([128, GROUP, KC, TILE_T], F16)
                nc.sync.dma_start(rep_sb[:], repT_d[g])
                scath_sb = scat_pool.tile([TILE_T, GROUP, TILE_T], F16, tag="scath")
                nc.sync.dma_start(scath_sb[:], scatH_d[g])
                if lab_bias:
                    scatl_sb = scat_pool.tile([NREL, GROUP, TILE_T], F16, tag="scatl")
                    nc.sync.dma_start(scatl_sb[:], scatL_d[g])
                o_sb = out_pool.tile([128, GROUP, DOUT], F32)

                for ti in range(GROUP):
                    i = g * GROUP + ti
                    # [proj_in | gate_in | gate_self] and self potential
                    psum_a = psum_pool.tile([128, DOUT + 2], F32, tag="pa")
                    psum_b = psum2_pool.tile([128, DOUT], F32, tag="pb")
                    for kc in range(KC):
                        first, last = kc == 0, kc == KC - 1
                        nc.tensor.matmul(psum_a[:], rep_sb[:, ti, kc, :], wa_sb[:, kc, :],
                                         start=first, stop=last)
                        nc.tensor.matmul(psum_b[:], rep_sb[:, ti, kc, :], ws_sb[:, kc, :],
                                         start=first, stop=last)

                    # head-gather (+ relation bias) via scatter matmuls; the
                    # last column gathers gate_self and is unused
                    src_sb = src_pool.tile([128, DOUT + 2], F16)
                    nc.scalar.activation(src_sb[:], psum_a[:, 0:DOUT + 2], AF.Copy)
                    psum_g = psum_pool.tile([128, DOUT + 2], F32, tag="pg")
                    nc.tensor.matmul(psum_g[:], scath_sb[:, ti, :], src_sb[:],
                                     start=True, stop=not lab_bias)
                    if lab_bias:
                        nc.tensor.matmul(psum_g[:], scatl_sb[:, ti, :], ball_sb[:],
                                         start=False, stop=True)

                    # gate weights: sigmoid(gate [+1 folded bias]) * msoft^2 * mask
                    w_in = small_pool.tile([128, 1], F32, tag="w_in")
                    nc.scalar.activation(w_in[:], psum_g[:, DOUT:DOUT + 1], AF.Sigmoid,
                                         bias=1.0 if gate_bias_one else 0.0)
                    w_self = small_pool.tile([128, 1], F32, tag="w_self")
                    nc.scalar.activation(w_self[:], psum_a[:, DOUT + 1:DOUT + 2], AF.Sigmoid)
                    w_in_f = small_pool.tile([128, 1], F32, tag="w_in_f")
                    nc.vector.tensor_mul(w_in_f[:], w_in[:], aux_sb[:, i, 0:1])
                    w_self_f = small_pool.tile([128, 1], F32, tag="w_self_f")
                    nc.vector.tensor_mul(w_self_f[:], w_self[:], aux_sb[:, i, 1:2])

                    # res = relu(in_pot*w_in + self_pot*w_self): the DVE reads
                    # only one PSUM operand per instruction, so stage self_pot
                    # through SBUF, then one fused gated-relu op
                    sp_sb = big_pool.tile([128, DOUT], F32, tag="sp")
                    nc.vector.tensor_copy(sp_sb[:], psum_b[:])
                    nc.vector._custom_dve(GATED_RELU, out=o_sb[:, ti, :],
                                          in0=psum_g[:, 0:DOUT], in1=sp_sb[:],
                                          s0=w_in_f[:], s1=w_self_f[:])

                # one batched output DMA per group on the ACT HWDGE ring
                # (inputs use the SP ring); dst iterated p-major to match src
                out_view = out_d[g * GROUP * TILE_T:(g + 1) * GROUP * TILE_T, :].rearrange(
                    "(i p) c -> p i c", p=TILE_T)
                nc.scalar.dma_start(out_view, o_sb[:])

    nc.compile()
    return nc


def prep_core_inputs(c, rep, adj_arc, adj_lab, adj_mask_in, adj_mask_loop, mask,
                     Wa, Ws, ball, ntiles: int = NTILES, lab_bias: bool = True):
    """Build the per-core in_map (host-side shard + layout prep)."""
    tok = ntiles * TILE_T
    ngroups = ntiles // GROUP
    sh = slice(c * SPC, (c + 1) * SPC)
    rep_s = np.ascontiguousarray(rep[sh]).reshape(SPC * L, DIN)[:tok]
    x = rep_s.reshape(ngroups, GROUP, TILE_T, KC, 128)      # [g, tile, t, kc, k]
    repT = np.ascontiguousarray(x.transpose(0, 4, 1, 3, 2).astype(NP_MM))  # [g, k, tile, kc, t]

    sent = adj_arc[sh, :, 0].reshape(-1)[:tok].astype(np.int64)
    head = adj_arc[sh, :, 1].reshape(-1)[:tok].astype(np.int64)
    idx_local = sent * L + head - c * SPC * L
    t_all = np.arange(tok)
    if idx_local.min() < 0 or idx_local.max() >= tok or np.any(idx_local // TILE_T != t_all // TILE_T):
        raise ValueError("head gather escapes its 128-token tile; unsupported input structure")

    scatH = np.zeros((ngroups, TILE_T, GROUP, TILE_T), NP_MM)
    scatH[t_all // (GROUP * TILE_T), idx_local % TILE_T,
          (t_all // TILE_T) % GROUP, t_all % TILE_T] = 1.0

    msq_in = (adj_mask_in[sh] ** 2 * mask[sh]).reshape(-1)[:tok].astype(np.float32)
    msq_loop = (adj_mask_loop[sh] ** 2 * mask[sh]).reshape(-1)[:tok].astype(np.float32)
    aux = np.ascontiguousarray(
        np.stack([msq_in.reshape(ntiles, TILE_T).T, msq_loop.reshape(ntiles, TILE_T).T], axis=-1)
    )  # [128, ntiles, 2]

    in_map = {"repT": repT, "scatH": scatH, "wa": Wa, "ws": Ws, "aux": aux}
    if lab_bias:
        lab = adj_lab[sh].reshape(-1)[:tok].astype(np.int64)
        scatL = np.zeros((ngroups, NREL, GROUP, TILE_T), NP_MM)
        scatL[t_all // (GROUP * TILE_T), lab, (t_all // TILE_T) % GROUP, t_all % TILE_T] = 1.0
        in_map["scatL"] = scatL
        in_map["ball"] = ball
    return in_map


def prep_shared(W_in, b_in, W_gate_in, b_gate_in, W_self, W_gate_self):
    Wa = np.concatenate([W_in, W_gate_in, W_gate_self], axis=1).astype(np.float32)
    Wa = np.ascontiguousarray(Wa.reshape(KC, 128, DOUT + 2).transpose(1, 0, 2).astype(NP_MM))
    Ws = np.ascontiguousarray(
        np.asarray(W_self, np.float32).reshape(KC, 128, DOUT).transpose(1, 0, 2).astype(NP_MM))
    ball = np.ascontiguousarray(np.concatenate(
        [b_in, b_gate_in, np.zeros((NREL, 1), np.float32)], axis=1).astype(NP_MM))
    return Wa, Ws, ball


_NC_CACHE = {}


def get_nc(lab_bias: bool, gate_bias_one: bool):
    key = (lab_bias, gate_bias_one)
    if key not in _NC_CACHE:
        _NC_CACHE[key] = build_nc(lab_bias=lab_bias, gate_bias_one=gate_bias_one)
    return _NC_CACHE[key]


def kernel(rep, adj_mask_in, adj_mask_loop, mask, W_in, b_in, W_gate_in,
           b_gate_in, W_self, W_gate_self, adj_arc_in, adj_lab_in):
    rep = np.asarray(rep, dtype=np.float32)
    b_in = np.asarray(b_in, dtype=np.float32)
    b_gate_in = np.asarray(b_gate_in, dtype=np.float32)
    # b_in == 0 makes the relation-bias gather a no-op; constant b_gate_in
    # folds into the sigmoid bias. setup_inputs always hits this path.
    lab_bias = not (np.all(b_in == 0.0) and np.all(b_gate_in == 1.0))
    Wa, Ws, ball = prep_shared(np.asarray(W_in), b_in, np.asarray(W_gate_in),
                               b_gate_in, np.asarray(W_self), np.asarray(W_gate_self))
    adj_arc = np.asarray(adj_arc_in)
    adj_lab = np.asarray(adj_lab_in)
    in_maps = [
        prep_core_inputs(c, rep, adj_arc, adj_lab, np.asarray(adj_mask_in),
                         np.asarray(adj_mask_loop), np.asarray(mask), Wa, Ws, ball,
                         lab_bias=lab_bias)
        for c in range(NCORES)
    ]

    nc = get_nc(lab_bias, gate_bias_one=not lab_bias)
    res = bass_utils.run_bass_kernel_spmd(nc, in_maps, core_ids=list(range(NCORES)))
    out = np.concatenate([r["out"].reshape(SPC, L, DOUT) for r in res.results], axis=0)
    return out
